# revision 51
# baseline (speedup 1.0000x reference)
"""Talking-heads attention with GFSA reaction term — TRN2 Bass kernel, 8 cores.

Sharding: (batch b, query-half) -> core c = b*2 + half. Each core handles all 12
heads for its 512 query rows; the key/value axis m stays full but is ROTATED on
the host so m-tiles 0..3 are the core's own query rows ("local") and 4..7 the
partner's. The only cross-core dependency is u = (1-2lam)v + 3lam*(attn3 @ v)
over the full query axis, exchanged with a per-pair AllGather (1.5MB) that
overlaps pass-2's local-half matmuls; the remote half is extracted with a
host-fed 0/1 mask blend so the program stays rank-symmetric.

Math (associativity rewrite — never materialize attn3 @ attn3):
  attn2[g]  = sum_h W1[g,h] (q_h*SCALE) @ k_h^T + b1[g]   (folded into QK^T)
  s_g       = softmax_m(attn2[g])
  attn3[g'] = sum_g W2[g',g] s_g + b2[g']                  (folded into A@v)
  w1[g']    = attn3[g'] @ v_g'
  out[g']   = attn3[g'] @ ((1-2*lam)v_g' + 3*lam*w1[g'])
  y         = out @ Wo^T + ob

The three heavy O(N^2*DIM) phases — mixed scores, pass 1 (attn3@v fold) and
pass 2 (attn3@u fold) — run as fp8e4 (e4m3) DoubleRow matmuls: 2 contraction
k-tiles per instruction at 2x the fp16 column rate. Operand scaling keeps every
fp8 tensor in e4m3's healthy range, with all compensation folded into host-side
tables:
  qsc = q*W1          (sigma~0.29; SCALE=1/8 moved into the Exp activation)
  kT  (sigma~1)
  E   = 64*softmax    (typ 0.06, max 64)
  Vt  = 8*3lam*W2*v   (sigma~0.7)  -> psA = 512*w1term, cL/u16 tables x512
  Ut  = (512u)*(W2/512) = u*W2 (sigma~0.29) -> psOut = 64*out
  outT = (psOut + b2uT)/64, output projection unchanged fp16.
Pass 1 accumulates all 12 g and all m-tiles into 8 long-lived PSUM bank groups
([n, d] orientation, feeds the u build). Pass 2 runs TRANSPOSED ([d, n]
orientation: lhsT=Ut, rhs=E) into 6 PSUM banks so the output-projection needs
no PE transposes. A junk-matmul warmup burst at t=0 overlaps the input DMAs
and lifts the PE HAM clock gate to 8/8 before real work starts.
"""
import numpy as np

import concourse.bacc as bacc
import concourse.mybir as mybir
import concourse.tile as tile
from concourse.bass_utils import run_bass_kernel_spmd

B, N, DIM, HEADS, HD = 4, 1024, 768, 12, 64
NH = N // 2                      # query rows per core
SCALE = HD ** -0.5
f32 = mybir.dt.float32
f16 = mybir.dt.float16
f8 = mybir.dt.float8e4
AL = mybir.AluOpType
AF = mybir.ActivationFunctionType
DR = mybir.MatmulPerfMode.DoubleRow

ESC = 64.0                       # E stored as ESC * softmax prob
VSC = 8.0                        # extra scale on Vt8 (=> psA,u16 carry ESC*VSC)
USC = ESC * VSC                  # u16L carries USC * u

TRACE = False                    # test.py may flip this for profiling
TRACE_KW = {}
DEBUG_DUMPS = False              # debug script flips this to dump intermediates


def _build():
    nc = bacc.Bacc("TRN2", target_bir_lowering=False, debug=False, num_devices=8)

    xf_T = nc.declare_dram_parameter("xf_T", [DIM, N], f16, isOutput=False)
    wq_T = nc.declare_dram_parameter("wq_T", [DIM, DIM], f8, isOutput=False)
    wk_T = nc.declare_dram_parameter("wk_T", [DIM, DIM], f8, isOutput=False)
    wv_T = nc.declare_dram_parameter("wv_T", [DIM, DIM], f16, isOutput=False)
    wo_T = nc.declare_dram_parameter("wo_T", [DIM, DIM], f16, isOutput=False)
    w1v = nc.declare_dram_parameter("w1v", [128, 72], f32, isOutput=False)
    b1bc = nc.declare_dram_parameter("b1bc", [128, HEADS], f32, isOutput=False)
    w2f = nc.declare_dram_parameter("w2f", [1, HEADS * DIM], f16, isOutput=False)
    w2fu = nc.declare_dram_parameter("w2fu", [1, HEADS * DIM], f16, isOutput=False)
    uc1 = nc.declare_dram_parameter("uc1", [1, DIM], f16, isOutput=False)
    b2blku = nc.declare_dram_parameter("b2blku", [1, DIM], f16, isOutput=False)
    b2blkT = nc.declare_dram_parameter("b2blkT", [128, 6], f32, isOutput=False)
    ob = nc.declare_dram_parameter("ob", [1, DIM], f16, isOutput=False)
    msk = nc.declare_dram_parameter("msk", [128, 2], f32, isOutput=False)
    y = nc.declare_dram_parameter("y", [NH, DIM], f32, isOutput=True)
    warm_out = nc.declare_dram_parameter("warm_out", [1, 8], f32, isOutput=True)
    if DEBUG_DUMPS:
        dbg_qT = nc.declare_dram_parameter("dbg_qT", [128, 6, NH], f16, isOutput=True)
        dbg_kT = nc.declare_dram_parameter("dbg_kT", [128, 6, N], f8, isOutput=True)
        dbg_qsc = nc.declare_dram_parameter("dbg_qsc", [128, 6, NH], f8, isOutput=True)
        dbg_Etmp = nc.declare_dram_parameter("dbg_Etmp", [128, 8, NH], f16, isOutput=True)
        dbg_E = nc.declare_dram_parameter("dbg_E", [128, 8, NH], f8, isOutput=True)
        dbg_u16L = nc.declare_dram_parameter("dbg_u16L", [128, 4, DIM], f16, isOutput=True)
        dbg_outT = nc.declare_dram_parameter("dbg_outT", [128, 6, NH], f16, isOutput=True)
        dbg_u16R = nc.declare_dram_parameter("dbg_u16R", [128, 4, DIM], f16, isOutput=True)
        dbg_b2uT = nc.declare_dram_parameter("dbg_b2uT", [128, 6], f32, isOutput=True)
        dbg_psO = nc.declare_dram_parameter("dbg_psO", [128, 6, NH], f32, isOutput=True)

    with tile.TileContext(nc) as tc:
        with tc.tile_pool(name="persist", bufs=1) as pp:
            # [m%128, m//128, g, n_local] fp8 64*probabilities (transposed
            # layout); written by the normalize pass from the fp16 staging.
            E = pp.tile([128, HEADS, 8, NH], f8)
            v16 = pp.tile([128, 8, DIM], f16)          # [m%128, m//128, (g',d)]
            w1v_sb = pp.tile([128, 72], f32)
            b1_sb = pp.tile([128, HEADS], f32)
            uc1bc = pp.tile([128, DIM], f16)
            b2tT = pp.tile([128, 6], f32)
            b2bcu = pp.tile([128, DIM], f16)
            msk_sb = pp.tile([128, 2], f32)
            ones128 = pp.tile([128, 128], f16)
            b2vu = pp.tile([128, DIM], f16)
            cL = pp.tile([128, 4, DIM], f16)
            VtA = pp.tile([128, 8, DIM], f8)   # pass-1 Vt for g=0, gpsimd-prebuilt
            w2all = pp.tile([128, HEADS, DIM], f16)
            w2allu = pp.tile([128, HEADS, DIM], f16)

            # ---- PE warmup: junk matmuls fill the input-DMA wait and lift the
            # HAM clock gate to 8/8 before the real stream begins. The tiny
            # copy-out keeps DCE away; warm_out is never read by the host.
            with tc.tile_pool(name="warm", bufs=1) as wp, \
                 tc.tile_pool(name="warmps", bufs=1, space="PSUM") as wps:
                wsrc = wp.tile([128, 512], f16)
                wdst = wp.tile([1, 8], f32)
                nc.vector.memset(wsrc[:], 1.0)
                pw = wps.tile([128, 512], f32)
                for _ in range(14):
                    nc.tensor.matmul(pw[:], wsrc[:, 0:128], wsrc[:],
                                     start=True, stop=True)
                nc.vector.tensor_copy(wdst[:], pw[0:1, 0:8])
                nc.scalar.dma_start(warm_out[:], wdst[:])

            nc.scalar.dma_start(w1v_sb[:], w1v[:])
            nc.scalar.dma_start(b1_sb[:], b1bc[:])
            nc.scalar.dma_start(msk_sb[:], msk[:])
            nc.scalar.dma_start(b2tT[:], b2blkT[:])
            nc.scalar.dma_start(b2bcu[:], b2blku[0:1, :].to_broadcast((128, DIM)))
            nc.vector.memset(ones128[:], 1.0)

            with tc.tile_pool(name="qk", bufs=1) as qk:
                qT = qk.tile([128, 6, NH], f16)        # [d%128, d//128, n_local]
                kT = qk.tile([128, 6, N], f8)          # [d%128, d//128, m]

                # ---- Phase A: QKV projections (all fp16; each weight matrix
                #      arrives as ONE large DMA so the PE never starves) ------
                with tc.tile_pool(name="pha", bufs=1) as pa, \
                     tc.tile_pool(name="psa", bufs=2, space="PSUM") as psa, \
                     tc.tile_pool(name="psav", bufs=1, space="PSUM") as psav:
                    xf = pa.tile([128, 6, N], f16)
                    xf8 = pa.tile([128, 6, N], f8)
                    wqs = pa.tile([128, 6, DIM], f8)
                    wks = pa.tile([128, 6, DIM], f8)
                    wvs = pa.tile([128, 6, DIM], f16)
                    xfr = xf_T.rearrange("(c p) n -> p c n", p=128)
                    wqr = wq_T.rearrange("(c p) n -> p c n", p=128)
                    wkr = wk_T.rearrange("(c p) n -> p c n", p=128)
                    wvr_ap = wv_T.rearrange("(c p) n -> p c n", p=128)
                    # pair-batched transfers: fewer ring-throttle stalls
                    # than per-d chunks, smaller than whole-tensor shots
                    for d2 in range(2):
                        dd = slice(2 * d2, 2 * d2 + 2)
                        nc.sync.dma_start(wqs[:, dd, :], wqr[:, dd, :])
                        nc.gpsimd.dma_start(xf[:, dd, 0:NH], xfr[:, dd, 0:NH])
                    # third local pair rides the scalar ring so the Q
                    # projection's full contraction lands ~5us earlier
                    nc.scalar.dma_start(xf[:, 4:6, 0:NH], xfr[:, 4:6, 0:NH])
                    nc.sync.dma_start(wqs[:, 4:6, :], wqr[:, 4:6, :])
                    for d2 in range(3):
                        dd = slice(2 * d2, 2 * d2 + 2)
                        nc.scalar.dma_start(wks[:, dd, :], wkr[:, dd, :])
                        nc.gpsimd.dma_start(xf[:, dd, NH:N], xfr[:, dd, NH:N])
                    for d2 in range(3):
                        dd = slice(2 * d2, 2 * d2 + 2)
                        nc.sync.dma_start(wvs[:, dd, :], wvr_ap[:, dd, :])
                    for d in range(6):
                        nc.vector.tensor_copy(xf8[:, d, :], xf[:, d, :])

                    for qc in range(6):                # qT[c, n] = sum_d wq[d,c]x[n,d]
                        ps = psa.tile([128, NH], f32, tag="ps512")
                        for d in range(3):
                            nc.tensor.matmul(ps[:],
                                             wqs[:, 2 * d:2 * d + 2,
                                                 qc * 128:(qc + 1) * 128],
                                             xf8[:, 2 * d:2 * d + 2, 0:NH],
                                             start=(d == 0), stop=(d == 2),
                                             perf_mode=DR)
                        nc.vector.tensor_copy(qT[:, qc, :], ps[:])
                    for kc in range(6):
                        for mc in range(2):
                            ps = psa.tile([128, 512], f32, tag="ps512")
                            for d in range(3):
                                nc.tensor.matmul(ps[:],
                                                 wks[:, 2 * d:2 * d + 2,
                                                     kc * 128:(kc + 1) * 128],
                                                 xf8[:, 2 * d:2 * d + 2,
                                                     mc * 512:(mc + 1) * 512],
                                                 start=(d == 0), stop=(d == 2),
                                                 perf_mode=DR)
                            nc.vector.tensor_copy(kT[:, kc, mc * 512:(mc + 1) * 512],
                                                  ps[:])
                    for vc in range(2):                # v[m, c] = sum_d x[m,d]wv[d,c]
                        for mt in range(8):
                            ps = psa.tile([128, 384], f32, tag="ps384")
                            for d in range(6):
                                nc.tensor.matmul(ps[:], xf[:, d, mt * 128:(mt + 1) * 128],
                                                 wvs[:, d, vc * 384:(vc + 1) * 384],
                                                 start=(d == 0), stop=(d == 5))
                            nc.vector.tensor_copy(v16[:, mt, vc * 384:(vc + 1) * 384],
                                                  ps[:])
                    # b2vu = USC*3lam*b2blk * colsum(v), row-replicated (all-
                    # ones lhsT); this is the attn3-bias part of w1, pre-scaled
                    # so the u16 build is a plain add.
                    psV = psav.tile([128, 2, 512], f32)
                    for half in range(2):
                        for mt in range(8):
                            nc.tensor.matmul(psV[:, half, 0:384],
                                             ones128[:],
                                             v16[:, mt, half * 384:(half + 1) * 384],
                                             start=(mt == 0), stop=(mt == 7))
                    nc.vector.tensor_tensor(b2vu[:, 0:384], psV[:, 0, 0:384],
                                            b2bcu[:, 0:384], AL.mult)
                    nc.vector.tensor_tensor(b2vu[:, 384:768], psV[:, 1, 0:384],
                                            b2bcu[:, 384:768], AL.mult)

                # ---- Phase B: mixed scores (mix1 fold) in fp8 DoubleRow, exp
                #      to fp16 staging, Z, normalize into fp8 E (x64) ---------
                with tc.tile_pool(name="qsc", bufs=2) as qscp, \
                     tc.tile_pool(name="etmp", bufs=3) as etp, \
                     tc.tile_pool(name="zbc", bufs=2) as zbcp, \
                     tc.tile_pool(name="psb", bufs=3, space="PSUM") as psb, \
                     tc.tile_pool(name="psz", bufs=2, space="PSUM") as psz:
                    for g in range(HEADS):
                        qsc = qscp.tile([128, 6, NH], f8, tag="qsc")
                        for i in range(6):
                            # split the W1-fold scalings across ACT and DVE so
                            # neither throttles phase B
                            if i % 2 == 0:
                                nc.scalar.activation(qsc[:, i, :], qT[:, i, :],
                                                     AF.Copy,
                                                     scale=w1v_sb[:, g * 6 + i:g * 6 + i + 1])
                            else:
                                nc.vector.tensor_scalar(qsc[:, i, :], qT[:, i, :],
                                                        w1v_sb[:, g * 6 + i:g * 6 + i + 1],
                                                        None, AL.mult)
                        Etmp = etp.tile([128, 8, NH], f16, tag="etmp")
                        for mt2 in range(4):
                            ps = psb.tile([128, 2, NH], f32, tag="psb")
                            for mc in range(2):
                                mt = 2 * mt2 + mc
                                for i in range(3):
                                    nc.tensor.matmul(
                                        ps[:, mc, :],
                                        kT[:, 2 * i:2 * i + 2, mt * 128:(mt + 1) * 128],
                                        qsc[:, 2 * i:2 * i + 2, :],
                                        start=(i == 0), stop=(i == 2), perf_mode=DR)
                            # psum holds 512x the true mixed score (x8 each
                            # from wq8/wk8 plus SCALE staying out of qsc)
                            nc.scalar.activation(Etmp[:, 2 * mt2:2 * mt2 + 2, :],
                                                 ps[:], AF.Exp,
                                                 bias=b1_sb[:, g:g + 1],
                                                 scale=1.0 / 512.0)
                        # Zrow_g[n] = sum_m E_g[m, n]  (free-axis layout, M=1)
                        psZ = psz.tile([1, NH], f32, tag="psz")
                        for mt in range(8):
                            nc.tensor.matmul(psZ[0:1, :], ones128[:, 0:1],
                                             Etmp[:, mt, :], start=(mt == 0),
                                             stop=(mt == 7))
                        # fast approx ESC/Z (Z is positive, O(100..3000)), cast
                        # fp16, then an on-chip partition broadcast — no DRAM
                        # bounce and no slow exact reciprocal
                        ztmp = zbcp.tile([1, NH], f32, tag="zt")
                        nc.vector.reciprocal_approx_fast(ztmp[0:1, :], psZ[0:1, :])
                        ztmp16 = zbcp.tile([1, NH], f16, tag="zt16")
                        nc.vector.tensor_scalar(ztmp16[0:1, :], ztmp[0:1, :],
                                                ESC, None, AL.mult)
                        zb = zbcp.tile([128, 1, NH], f16, tag="zb")
                        nc.gpsimd.partition_broadcast(zb[:, 0, :], ztmp16[0:1, :])
                        for mt2 in range(4):
                            nc.vector.tensor_tensor(
                                E[:, g, 2 * mt2:2 * mt2 + 2, :],
                                Etmp[:, 2 * mt2:2 * mt2 + 2, :],
                                zb[:].to_broadcast((128, 2, NH)), AL.mult)
                        if g in (9, 10):
                            # prebuild pass-1 Vt(g'=0) on the idle gpsimd
                            # queue at the phase-B tail so pass 1 opens
                            # matmul-hot (2 pair-builds per head keeps any
                            # zb-broadcast delay under ~4us)
                            for t in range(2 * (g - 9), 2 * (g - 9) + 2):
                                nc.gpsimd.tensor_tensor(
                                    VtA[:, 2 * t:2 * t + 2, :],
                                    v16[:, 2 * t:2 * t + 2, :],
                                    w2allu[:, 0:1, :].to_broadcast((128, 2, DIM)),
                                    AL.mult)
                        if g == 8:
                            # w2all/w2allu broadcasts: late enough that their
                            # ~5MB of SBUF writes only overlap the phase-B
                            # tail, early enough to finish before pass 1.
                            # Gated on v16 via the 1-elem copies; issued from
                            # the sync queue so the ACT stream is untouched.
                            nc.vector.tensor_copy(uc1bc[0:1, 0:1],
                                                  v16[0:1, 0, 0:1])
                            nc.vector.tensor_copy(b2bcu[0:1, 0:1],
                                                  v16[0:1, 0, 0:1])
                            nc.vector.tensor_copy(w2allu[0:1, 0, 0:1],
                                                  v16[0:1, 0, 0:1])
                            nc.vector.tensor_copy(w2all[0:1, 0, 0:1],
                                                  v16[0:1, 0, 0:1])
                            nc.sync.dma_start(uc1bc[:], uc1[0:1, :].to_broadcast((128, DIM)))
                            for gg in range(HEADS):
                                nc.sync.dma_start(
                                    w2allu[:, gg, :],
                                    w2fu[0:1, gg * DIM:(gg + 1) * DIM].to_broadcast((128, DIM)))
                            for gg in range(HEADS):
                                nc.sync.dma_start(
                                    w2all[:, gg, :],
                                    w2f[0:1, gg * DIM:(gg + 1) * DIM].to_broadcast((128, DIM)))
                        if DEBUG_DUMPS and g == 0:
                            nc.sync.dma_start(dbg_qT[:], qT[:])
                            nc.sync.dma_start(dbg_kT[:], kT[:])
                            nc.sync.dma_start(dbg_qsc[:], qsc[:])
                            nc.sync.dma_start(dbg_Etmp[:], Etmp[:])
                            for mt in range(8):
                                nc.sync.dma_start(dbg_E[:, mt, :], E[:, 0, mt, :])

            with tc.tile_pool(name="late", bufs=1) as late, \
                 tc.tile_pool(name="phf", bufs=1) as pf, \
                 tc.tile_pool(name="wos", bufs=6) as wos:
                u16L = late.tile([128, 4, DIM], f16)
                u16R = late.tile([128, 4, DIM], f16)
                # cL[j] = USC*((1-2lam)*v + 3lam*b2v)  — everything of u16L
                # except the pass-1 matmul result; built early, off the
                # critical path
                for j in range(4):
                    nc.vector.tensor_tensor(cL[:, j, :], v16[:, j, :],
                                            uc1bc[:], AL.mult)
                for j in range(4):
                    nc.vector.tensor_tensor(cL[:, j, :], cL[:, j, :],
                                            b2vu[:], AL.add)

                # ---- Pass 1: 3lam*(attn3 @ v) via the host-folded 8*3lam*W2
                #      table, fp8 DoubleRow (8 PSUM bank groups accumulate over
                #      all g, m-tile pairs) -----------------------------------
                with tc.tile_pool(name="vt", bufs=2) as vtp, \
                     tc.tile_pool(name="psc", bufs=4, space="PSUM") as psc, \
                     tc.tile_pool(name="pscb", bufs=2, space="PSUM") as pscb:
                    psAs = [psc.tile([128, 512], f32, tag="pscA", name=f"pscA{i}") for i in range(4)]
                    # the four 1KB psB accumulation groups pack pairwise into 2
                    # banks: only the even member ever issues start=True (which
                    # zeroes the whole 2KB bank — incl. the odd sibling, whose
                    # first accumulate comes later in PE program order).
                    psBp = [pscb.tile([128, 2, 256], f32, tag="pscB", name=f"pscB{i}") for i in range(2)]
                    psBs = [psBp[0][:, 0, :], psBp[0][:, 1, :],
                            psBp[1][:, 0, :], psBp[1][:, 1, :]]
                    for g in range(HEADS):
                        if g == 0:
                            Vt = VtA                # prebuilt during phase B
                        else:
                            Vt = vtp.tile([128, 8, DIM], f8, tag="vt")
                            for t in range(4):
                                # pairwise [128,2,768] build: one op per DR
                                # slab pair, in1 stride-0-broadcast
                                nc.vector.tensor_tensor(
                                    Vt[:, 2 * t:2 * t + 2, :],
                                    v16[:, 2 * t:2 * t + 2, :],
                                    w2allu[:, g:g + 1, :].to_broadcast((128, 2, DIM)),
                                    AL.mult)
                        for ns in range(4):
                            for t in range(4):
                                lhs = E[:, g, 2 * t:2 * t + 2,
                                        ns * 128:(ns + 1) * 128]
                                first = (g == 0 and t == 0)
                                last = (g == HEADS - 1 and t == 3)
                                nc.tensor.matmul(psAs[ns][:], lhs,
                                                 Vt[:, 2 * t:2 * t + 2, 0:512],
                                                 start=first, stop=last,
                                                 perf_mode=DR)
                                nc.tensor.matmul(psBs[ns], lhs,
                                                 Vt[:, 2 * t:2 * t + 2, 512:768],
                                                 start=(first and ns % 2 == 0),
                                                 stop=last, perf_mode=DR,
                                                 skip_group_check=(ns % 2 == 1))

                # ---- u16L = cL + pass1, AllGather u + pass 2 ----------------
                # u16L ships per-j so the collective fires ~2us after pass 1's
                # last matmul. The partner block needs only a rank-symmetric
                # mask blend. Pass 2 runs TRANSPOSED: psOut[cb] = [d-block, n]
                # so the output projection consumes it with no PE transposes.
                outT = late.tile([128, 6, NH], f16)
                b2uT = late.tile([128, 6], f32)
                with tc.tile_pool(name="dram", bufs=1, space="DRAM") as dram, \
                     tc.tile_pool(name="w1g", bufs=1) as w1gp, \
                     tc.tile_pool(name="ut", bufs=2) as utp, \
                     tc.tile_pool(name="pse", bufs=6, space="PSUM") as pse, \
                     tc.tile_pool(name="psu", bufs=1, space="PSUM") as psu:
                    # row NH carries colsum(u16L) so one AllGather moves both
                    u16loc = dram.tile([NH + 1, DIM], f16)
                    u16full = dram.tile([2 * (NH + 1), DIM], f16)
                    u16locr = u16loc[0:NH, :].rearrange("(ns p) j -> p ns j", p=128)
                    psU = psu.tile([128, 2, 512], f32)
                    for j in range(4):
                        nc.vector.tensor_tensor(u16L[:, j, 0:512], psAs[j][:],
                                                cL[:, j, 0:512], AL.add)
                        nc.vector.tensor_tensor(u16L[:, j, 512:768], psBs[j],
                                                cL[:, j, 512:768], AL.add)
                        nc.sync.dma_start(u16locr[:, j, :], u16L[:, j, :])
                        # local colsum(u) rides along with the AllGather, so
                        # the bias term needs no post-collective matmuls
                        nc.tensor.matmul(psU[:, 0, :], ones128[:], u16L[:, j, 0:512],
                                         start=(j == 0), stop=(j == 3))
                        nc.tensor.matmul(psU[:, 1, 0:256], ones128[:],
                                         u16L[:, j, 512:768],
                                         start=(j == 0), stop=(j == 3))
                    if DEBUG_DUMPS:
                        nc.sync.dma_start(dbg_u16L[:], u16L[:])
                    # psU holds USC*colsum_u = 512*colsum_u which can overflow
                    # fp16; ship 8*colsum_u instead (b2tT compensates with x8)
                    cs_sb = w1gp.tile([1, DIM], f16)
                    nc.vector.tensor_scalar(cs_sb[0:1, 0:512], psU[0:1, 0, :],
                                            1.0 / ESC, None, AL.mult)
                    nc.vector.tensor_scalar(cs_sb[0:1, 512:768], psU[0:1, 1, 0:256],
                                            1.0 / ESC, None, AL.mult)
                    nc.sync.dma_start(u16loc[NH:NH + 1, :], cs_sb[0:1, :])
                    nc.gpsimd.collective_compute(
                        "AllGather", AL.bypass,
                        replica_groups=[[0, 1], [2, 3], [4, 5], [6, 7]],
                        ins=[u16loc.opt()], outs=[u16full.opt()])
                    psOut = [pse.tile([128, 512], f32, tag="psO", name=f"psO{i}")
                             for i in range(6)]
                    fence = w1gp.tile([128, 1], f32)
                    for g in range(HEADS):
                        UtL = utp.tile([128, 4, DIM], f8, tag="ut")
                        for jp in range(2):
                            nc.vector.tensor_tensor(
                                UtL[:, 2 * jp:2 * jp + 2, :],
                                u16L[:, 2 * jp:2 * jp + 2, :],
                                w2all[:, g:g + 1, :].to_broadcast((128, 2, DIM)),
                                AL.mult)
                            for cb in range(6):
                                nc.tensor.matmul(
                                    psOut[cb][:],
                                    UtL[:, 2 * jp:2 * jp + 2,
                                        cb * 128:(cb + 1) * 128],
                                    E[:, g, 2 * jp:2 * jp + 2, :],
                                    start=(g == 0 and jp == 0), stop=False,
                                    perf_mode=DR)
                        if g == HEADS - 1:
                            nc.vector.tensor_scalar(fence[:], UtL[:, 3, 0:1],
                                                    0.0, 1.0, AL.mult, AL.add)
                    # remote half: mask-blend of the two gathered blocks.
                    # The fence (==1.0, data-dependent on the last local UtL
                    # build) pins the blend AFTER all local Ut builds in the
                    # DVE stream so local work never head-of-line blocks on
                    # collective data:
                    #   u16R = ((b0*fence) - b1)*m0 + b1   (m0, m1=1-m0)
                    u16b = w1gp.tile([128, 2, 4, DIM], f16)
                    for blk in range(2):
                        blo = blk * (NH + 1)
                        nc.sync.dma_start(
                            u16b[:, blk, :, :],
                            u16full[blo:blo + NH, :].rearrange(
                                "(mt p) j -> p mt j", p=128))
                    nc.vector.scalar_tensor_tensor(
                        u16b[:, 0, :, :], u16b[:, 0, :, :], fence[:, 0:1],
                        u16b[:, 1, :, :], AL.mult, AL.subtract)
                    nc.vector.scalar_tensor_tensor(
                        u16R[:], u16b[:, 0, :, :], msk_sb[:, 0:1],
                        u16b[:, 1, :, :], AL.mult, AL.add)
                    # attn3-bias term: both ranks' colsum(u) rows arrived with
                    # the gather; transpose-load the [2, 768] block to
                    # [d%128, d//128, blk], sum the two blocks, scale by the
                    # host-transposed b2 table.
                    csT = w1gp.tile([128, 6, 2], f16)
                    for blk in range(2):
                        cro = blk * (NH + 1) + NH
                        nc.gpsimd.dma_start(
                            csT[:, :, blk:blk + 1],
                            u16full[cro:cro + 1, :].rearrange(
                                "o (c p) -> p c o", p=128))
                    nc.gpsimd.tensor_tensor(b2uT[:], csT[:, :, 0], csT[:, :, 1],
                                            AL.add)
                    nc.gpsimd.tensor_tensor(b2uT[:], b2uT[:], b2tT[:], AL.mult)

                    for g in range(HEADS):
                        UtR = utp.tile([128, 4, DIM], f8, tag="ut")
                        for jp in range(2):
                            nc.vector.tensor_tensor(
                                UtR[:, 2 * jp:2 * jp + 2, :],
                                u16R[:, 2 * jp:2 * jp + 2, :],
                                w2all[:, g:g + 1, :].to_broadcast((128, 2, DIM)),
                                AL.mult)
                            for cb in range(6):
                                nc.tensor.matmul(
                                    psOut[cb][:],
                                    UtR[:, 2 * jp:2 * jp + 2,
                                        cb * 128:(cb + 1) * 128],
                                    E[:, g, 4 + 2 * jp:4 + 2 * jp + 2, :],
                                    start=False,
                                    stop=(g == HEADS - 1 and jp == 1),
                                    perf_mode=DR)

                    # pass-2 consume: add the attn3-bias term (per-partition in
                    # the transposed layout) and undo the ESC scale, straight
                    # into outT
                    for cb in range(6):
                        nc.vector.tensor_scalar(outT[:, cb, :], psOut[cb][:],
                                                b2uT[:, cb:cb + 1], 1.0 / ESC,
                                                AL.add, AL.mult)
                    if DEBUG_DUMPS:
                        nc.sync.dma_start(dbg_outT[:], outT[:])
                        nc.sync.dma_start(dbg_u16R[:], u16R[:])
                        nc.sync.dma_start(dbg_b2uT[:], b2uT[:])
                        psO_sb = late.tile([128, 6, NH], f32)
                        for cb in range(6):
                            nc.vector.tensor_copy(psO_sb[:, cb, :], psOut[cb][:])
                        nc.sync.dma_start(dbg_psO[:], psO_sb[:])

                # ---- Phase F: output projection (outT fed directly) ---------
                with tc.tile_pool(name="ypool", bufs=2) as ypool:
                    obbc = pf.tile([128, DIM], f16)
                    nc.vector.tensor_copy(obbc[0:1, 0:1], v16[0:1, 0, 0:1])
                    nc.gpsimd.dma_start(obbc[:], ob[0:1, :].to_broadcast((128, DIM)))
                    wor = wo_T.rearrange("(c p) n -> p c n", p=128)
                    wo_ts = []
                    for jc in range(6):
                        wo_t = wos.tile([128, DIM], f16, tag="wo", name=f"wo{jc}")
                        nc.vector.tensor_copy(wo_t[0:1, 0:1], v16[0:1, 0, 0:1])
                        nc.gpsimd.dma_start(wo_t[:], wor[:, jc, :])
                        wo_ts.append(wo_t)
                    yr = y.rearrange("(ns p) j -> p ns j", p=128)
                    with tc.tile_pool(name="psf", bufs=2, space="PSUM") as psf:
                        for ns in range(4):
                            psY = psf.tile([128, 512], f32, tag="psY")
                            psY2 = psf.tile([128, 512], f32, tag="psY2")
                            for jc in range(6):
                                nc.tensor.matmul(psY[:, :],
                                                 outT[:, jc, ns * 128:(ns + 1) * 128],
                                                 wo_ts[jc][:, 0:512], start=(jc == 0),
                                                 stop=(jc == 5))
                                nc.tensor.matmul(psY2[:, 0:256],
                                                 outT[:, jc, ns * 128:(ns + 1) * 128],
                                                 wo_ts[jc][:, 512:768], start=(jc == 0),
                                                 stop=(jc == 5))
                            y_sb = ypool.tile([128, DIM], f32, tag="ysb")
                            nc.vector.tensor_tensor(y_sb[:, 0:512], psY[:, :],
                                                    obbc[:, 0:512], AL.add)
                            nc.vector.tensor_tensor(y_sb[:, 512:768], psY2[:, 0:256],
                                                    obbc[:, 512:768], AL.add)
                            nc.sync.dma_start(yr[:, ns, :], y_sb[:])

    nc.compile()
    return nc


def kernel(x, qkv_w, proj_l_w, proj_l_b, proj_w_w, proj_w_b, lamb,
           proj_out_w, proj_out_b):
    x = np.asarray(x, dtype=np.float32)
    qkv_w = np.asarray(qkv_w, dtype=np.float32)
    proj_l_w = np.asarray(proj_l_w, dtype=np.float32)
    proj_l_b = np.asarray(proj_l_b, dtype=np.float32)
    proj_w_w = np.asarray(proj_w_w, dtype=np.float32)
    proj_w_b = np.asarray(proj_w_b, dtype=np.float32)
    lamb = np.asarray(lamb, dtype=np.float32)
    proj_out_w = np.asarray(proj_out_w, dtype=np.float32)
    proj_out_b = np.asarray(proj_out_b, dtype=np.float32)

    nc = _build()

    # NOTE: wq/wk ship as e4m3 scaled x8 (sigma ~0.29, clear of subnormals);
    # q and k then carry x8 each, and SCALE (1/8) also stays out of the
    # weights — the Exp activation's 1/512 scale undoes all of it.
    import ml_dtypes
    wq_T = np.ascontiguousarray(qkv_w[:DIM].T * 8.0).astype(ml_dtypes.float8_e4m3)
    wk_T = np.ascontiguousarray(qkv_w[DIM:2 * DIM].T * 8.0).astype(ml_dtypes.float8_e4m3)
    wv_T = np.ascontiguousarray(qkv_w[2 * DIM:].T).astype(np.float16)
    wo_T = np.ascontiguousarray(proj_out_w.T).astype(np.float16)

    w1v = np.empty((128, 72), dtype=np.float32)
    for g in range(HEADS):
        for i in range(6):
            w1v[:64, g * 6 + i] = proj_l_w[g, 2 * i]
            w1v[64:, g * 6 + i] = proj_l_w[g, 2 * i + 1]
    b1bc = np.tile(proj_l_b[None, :], (128, 1)).astype(np.float32)
    # w2f[0, g*768 + g'*64 + d] = proj_w_w[g', g] / USC  (u16 carries USC*u so
    # Ut8 = u*W2 lands in e4m3 range; psOut then carries ESC*out)
    w2f_master = np.repeat(proj_w_w.T, HD, axis=1).reshape(1, HEADS * DIM).astype(np.float32)
    w2f = (w2f_master / np.float32(USC)).astype(np.float16)
    lam_rep = np.repeat(3.0 * lamb, HD)[None, :].astype(np.float32)   # [1, 768]
    # Vt8 = VSC * 3lam*W2 * v
    w2fu = (w2f_master.reshape(HEADS, DIM) * lam_rep * np.float32(VSC)).reshape(
        1, HEADS * DIM).astype(np.float16)
    uc1 = (np.repeat(1.0 - 2.0 * lamb, HD)[None, :] * USC).astype(np.float16)
    b2blk = np.repeat(proj_w_b, HD)[None, :].astype(np.float32)
    b2blku = (b2blk * lam_rep * np.float32(USC)).astype(np.float16)
    # b2uT = (VSC*colsum_u) * b2tT must equal ESC*bias => b2tT = b2 * VSC
    b2blkT = np.ascontiguousarray((b2blk * np.float32(VSC)).reshape(6, 128).T
                                  ).astype(np.float32)
    ob = proj_out_b[None, :].astype(np.float16)

    in_maps = []
    for c in range(8):
        b, half = c // 2, c % 2
        # m-axis rotated: rows [0:512] are this core's own query rows
        xr = np.concatenate([x[b, half * NH:(half + 1) * NH, :],
                             x[b, (1 - half) * NH:(2 - half) * NH, :]], axis=0)
        mskv = np.empty((128, 2), dtype=np.float32)
        mskv[:, 0] = float(half)        # weight for gathered block 0 (= rank 0)
        mskv[:, 1] = float(1 - half)    # weight for gathered block 1 (= rank 1)
        in_maps.append({
            "xf_T": np.ascontiguousarray(xr.T).astype(np.float16),
            "wq_T": wq_T, "wk_T": wk_T, "wv_T": wv_T, "wo_T": wo_T,
            "w1v": w1v, "b1bc": b1bc, "w2f": w2f, "w2fu": w2fu,
            "uc1": uc1, "b2blku": b2blku, "b2blkT": b2blkT, "ob": ob, "msk": mskv,
        })

    res = run_bass_kernel_spmd(nc, in_maps, core_ids=list(range(8)),
                               trace=TRACE, **TRACE_KW)
    kernel.last_results = res
    kernel.last_nc = nc
    kernel.last_in_maps = in_maps

    out = np.empty((B, N, DIM), dtype=np.float32)
    for c in range(8):
        b, half = c // 2, c % 2
        out[b, half * NH:(half + 1) * NH, :] = res.results[c]["y"]
    return out
